# revision 36
# baseline (speedup 1.0000x reference)
"""Trainium2 Bass kernel for nn_Decoder_3539053052044 (v9, 569us vs 896us
baseline in TimelineSim; HW-verified rel err ~0.006).

Structure:
- The reference decoder has a preserved bug: every layer consumes the ORIGINAL
  x0, so only the LAST layer's output survives. We compute layer L-1 only.
- Sequence-parallel with ZERO cross-core communication: 8 cores x 256 tokens
  (core r -> batch r//4, chunk r%4). Each core computes the full last layer
  for its 256 tokens (K/V projections for its whole batch are computed
  locally), then projects its OWN 256 tokens against the FULL 32000-col vocab
  matrix, streamed from HBM in 64 x 1MB chunks (software-pipelined 4 deep on
  the gpsimd SWDGE queue, which does not hold its sequencer through the
  transfer). No AllGather / collective at all (the sim prices a collective at
  15us fixed + 40GB/s); the host concatenates row shards.
- x0fm is per-core TOKEN-ROTATED (own chunk first) so the q/residual chunk is
  the uniform slice x0_b[:, :, 0:TOK]; self-attn keys are in rotated order,
  compensated by rotating the per-core mask data identically.
- ALL large weights stream through one 4-slot rolling arena (64KB/partition)
  in need order: sWq,sWk,sWv,cWk,cWv,sWo,cWq,cWo,fW1x4,fW2x4. The tile-ring
  WAR dependency throttles each load behind the 4-back weight's last read,
  which is always long past - so weight DMA never stalls the PE and no
  per-phase pool address reuse stalls exist. fW1 is host-pre-split into
  column blocks (block j serves FFN1 m-tiles 8j..8j+7); FFN2 runs k-outer
  over fW2 row blocks (8 psum banks) so each block is consumed as it lands.
- Activations are feature-major [D on partitions, tokens free] so every linear
  layer uses the stored [D_in, D_out] weights directly as lhsT.
- Softmax is max-free; exp runs directly on the scoresT PSUM (ACT engine) and
  the additive mask is applied MULTIPLICATIVELY after exp (exp(s+m) =
  exp(s)*exp(m); host precomputes exp(mask) as bf16, exactly 0/1 for the
  causal mask). The bf16*bf16 multiply runs in the DVE 4x perf mode.
- V carries a ones column per head ([128, H*65]) so the AV matmul's 65th
  output row is the softmax denominator for free.
- LayerNorm runs feature-major via ones-matmul partition reductions; rstd is
  exp(-0.5*ln(var+eps)) so the single ACT table set
  (natural_log_exp_and_others: ln/exp/relu/identity) never reloads. The
  elementwise chains alternate DVE/GpSimd (GpSimd cannot touch PSUM, so the
  mean/rstd broadcasts are staged to SBUF first).
- EVERY matmul input is bf16 (fp32 lhsT costs 4 cycles/row in the sim and on
  HW): residual tiles, LN stats inputs, softmax reciprocals and the ones
  vectors are all bf16; accumulation stays fp32 in PSUM. Output rows are
  written bf16 and converted to f32 on the host. Measured end-to-end rel err
  ~0.006 (budget 2e-2).
"""

import numpy as np
import ml_dtypes

import concourse.bass as bass
import concourse.bacc as bacc
import concourse.tile as tile
from concourse import mybir
from concourse.bass_utils import run_bass_kernel_spmd
from concourse.vector_clock import ScopedClock, VectorClock

BF16 = ml_dtypes.bfloat16
F32 = mybir.dt.float32
BF = mybir.dt.bfloat16
PSUM = bass.MemorySpace.PSUM

B, S, D, H, L, V, DF = 2, 1024, 1024, 16, 4, 32000, 4096
DH = D // H              # 64
NC = 8                   # cores
TOK = B * S // NC        # 256 tokens per core
KT = S // 128            # 8 k tiles
FT = D // 128            # 8 feature tiles
HT = DF // 128           # 32 hidden tiles
VC = 500                 # vocab cols per chunk
VN = V // VC             # 64 vocab chunks
VG = 4                   # chunks per output-dma group
ADD = mybir.AluOpType.add
MULT = mybir.AluOpType.mult
IDENT = mybir.ActivationFunctionType.Identity

_PATCHED = False


def _patch_tile_drain():
    """This neuronxcc build rejects a Drain carrying >1 sem wait. Split the
    Tile tail drain into one Drain per busy proc, each with a single wait."""
    global _PATCHED
    if _PATCHED:
        return
    _PATCHED = True

    def _drain_and_barrier_split(self, tick_clock, wait_clock):
        gc = tick_clock.global_clock
        n = len(gc)
        for p in range(n):
            if gc[p] > 0:
                vc = VectorClock([gc[q] if q == p else 0 for q in range(n)])
                d = self.nc.sync.drain()
                wait_clock.add_sem_waits(d.ins, ScopedClock({None: vc}))
        self.nc.sync.drain()
        self.nc.all_engine_barrier()
        assert self.sems is not None
        popped = self.nc._tile_sem_poison_stack.pop()
        assert popped is self._sem_poison
        self.nc.clear_and_free_semaphores(list(self.sems.allocated().values()))
        self.nc.all_engine_barrier()

    tile.TileContext._drain_and_barrier = _drain_and_barrier_split


def positional_encoding(seq_len, d_model, n=10000.0):
    i = np.arange(seq_len, dtype=np.float32)[:, None]
    d = np.arange(d_model)
    denom = np.power(n, (2 * (d // 2)).astype(np.float32) / d_model)
    ang = i / denom
    return np.where(d % 2 == 0, np.sin(ang), np.cos(ang)).astype(np.float32)


BIAS_NAMES = ['sbk', 'sbq', 'sbo', 'cbk', 'cbq', 'cbo', 'fb2',
              'ln1_g', 'ln1_b', 'ln2_g', 'ln2_b', 'ln3_g', 'ln3_b']


def build_program(self_mask_mul: bool, cross_mask_mul: bool, zero_free_biases: bool = False,
                  zero_ln: bool = False, stop_phase: int = 99):
    _patch_tile_drain()
    nc = bacc.Bacc()

    g = {}  # dram handles
    # x0fm is per-core TOKEN-ROTATED (own chunk first) so the q/residual
    # chunk is a uniform [:, 0:TOK] slice; self-attn keys are then in rotated
    # order, compensated by rotating the m01 mask data identically.
    g['x0fm'] = nc.declare_dram_parameter("x0fm", [D, S], BF, isOutput=False)
    g['encfm'] = nc.declare_dram_parameter("encfm", [D, S], BF, isOutput=False)
    for w in ['sWq', 'sWk', 'sWv', 'sWo', 'cWq', 'cWk', 'cWv', 'cWo']:
        g[w] = nc.declare_dram_parameter(w, [D, D], BF, isOutput=False)
    # fW1 pre-split into 4 column blocks [128, FT, 1024] so FFN1 m-tiles
    # 8j..8j+7 only need block j; fW2 kept row-major (k-outer FFN2).
    g['fW1S'] = nc.declare_dram_parameter("fW1S", [128, 4 * FT * 1024], BF, isOutput=False)
    g['fW2'] = nc.declare_dram_parameter("fW2", [DF, D], BF, isOutput=False)
    g['WoutS'] = nc.declare_dram_parameter("WoutS", [128, VN * FT * VC], BF, isOutput=False)
    g['biases'] = nc.declare_dram_parameter("biases", [128, 8 * len(BIAS_NAMES)], F32, isOutput=False)
    g['fb1'] = nc.declare_dram_parameter("fb1", [128, HT], F32, isOutput=False)
    g['sbv_row'] = nc.declare_dram_parameter("sbv_row", [1, D], F32, isOutput=False)
    g['cbv_row'] = nc.declare_dram_parameter("cbv_row", [1, D], F32, isOutput=False)
    g['bout_row'] = nc.declare_dram_parameter("bout_row", [1, V], F32, isOutput=False)
    g['m01'] = nc.declare_dram_parameter("m01", [S, 2 * TOK], BF, isOutput=False) if self_mask_mul else None
    g['m01c'] = nc.declare_dram_parameter("m01c", [S, 2 * TOK], BF, isOutput=False) if cross_mask_mul else None
    g['out'] = nc.declare_dram_parameter("out", [TOK, V], BF, isOutput=True)

    with tile.TileContext(nc) as tc:
        _emit(nc, tc, g, zero_free_biases, zero_ln, stop_phase)
    nc.compile()
    return nc


class _StopEmit(Exception):
    pass


def _emit(nc, tc, g, zero_free_biases, zero_ln, stop_phase=99):
    try:
        _emit_inner(nc, tc, g, zero_free_biases, zero_ln, stop_phase)
    except _StopEmit:
        pass


def _emit_inner(nc, tc, g, zero_free_biases, zero_ln, stop_phase):
    def phase_gate(p):
        if stop_phase < p:
            raise _StopEmit()
    from contextlib import ExitStack
    ctx = ExitStack()
    with ctx:
        # ---------- whole-kernel constants / small tensors ------------------
        const = ctx.enter_context(tc.tile_pool(name="const", bufs=1))
        ones_f32 = const.tile([128, 1], F32, name="ones_f32", tag="c1")
        nc.gpsimd.memset(ones_f32[:], 1.0)
        ones_row = const.tile([1, 128], F32, name="ones_row", tag="c2")
        nc.gpsimd.memset(ones_row[:], 1.0)
        ones_col_bf = const.tile([128, 1], BF, name="ones_col_bf", tag="c7")
        nc.gpsimd.memset(ones_col_bf[:], 1.0)
        ones_row_bf = const.tile([1, 128], BF, name="ones_row_bf", tag="c8")
        nc.gpsimd.memset(ones_row_bf[:], 1.0)
        eps_t = const.tile([1, 1], F32, name="eps_t", tag="c5")
        nc.gpsimd.memset(eps_t[:], 1e-5)
        neghalf_t = const.tile([1, 1], F32, name="neghalf_t", tag="c6")
        nc.gpsimd.memset(neghalf_t[:], -0.5)
        bias_sb = const.tile([128, 8 * len(BIAS_NAMES)], F32, name="bias_sb", tag="c3")
        nc.scalar.dma_start(bias_sb[:], g['biases'][:])
        fb1_sb = const.tile([128, HT], F32, name="fb1_sb", tag="c4")
        nc.scalar.dma_start(fb1_sb[:], g['fb1'][:])
        def bias_col(name, f):
            i = BIAS_NAMES.index(name)
            return bias_sb[:, i * 8 + f:i * 8 + f + 1]

        # free-axis bias broadcast tiles [128, D] for sbv / cbv (skipped when
        # the host observed all-zero free-axis biases)
        free_bias = {'sbv': None, 'cbv': None}
        if not zero_free_biases:
            with tc.tile_pool(name="bbc_ps", bufs=1, space=PSUM) as bps, \
                 tc.tile_pool(name="bbc_row", bufs=2) as brow:
                for bi, bname in enumerate(['sbv', 'cbv']):
                    t = const.tile([128, D], F32, name=f"{bname}_b", tag=f"fb{bi}")
                    rsb = brow.tile([1, D], F32, tag="row")
                    nc.sync.dma_start(rsb[:], g[f'{bname}_row'][:])
                    for half in range(2):
                        ps = bps.tile([128, 512], F32, tag="bc")
                        nc.tensor.matmul(ps[:], ones_row[:],
                                         rsb[0:1, half * 512:(half + 1) * 512],
                                         start=True, stop=True)
                        nc.vector.tensor_copy(t[:, half * 512:(half + 1) * 512], ps[:])
                    free_bias[bname] = t

        def copy_out(ot, ps, idx):
            """psum->sbuf copy alternating DVE/ACT to balance engines"""
            if idx % 2 == 0:
                nc.vector.tensor_copy(ot, ps)
            else:
                nc.scalar.activation(ot, ps, IDENT)

        # Rolling weight arena: every [128*FT, 1024]-shaped weight block
        # streams through one 4-slot ring (64KB/partition). Slot n+4 reuses
        # slot n's space; the tile ring's WAR dep throttles each load until
        # the old block's last matmul read — with this need-ordered stream
        # every load has a >20us window, so the PE never waits on weights.
        _arena = {'pool': None, 'n': 0}

        def arena_load(dram_view):
            wt = _arena['pool'].tile([128, FT, 1024], BF,
                                     name=f"aw_{_arena['n']}", tag="w")
            eng = nc.sync if _arena['n'] % 2 == 0 else nc.gpsimd
            _arena['n'] += 1
            eng.dma_start(wt[:], dram_view)
            return wt

        def arena_load_w(w_name):
            wt = arena_load(g[w_name].rearrange("(a p) d -> p a d", p=128)[:])

            def wslice(k, c0, c1):
                return wt[:, k, c0:c1]
            return wslice

        # LN outputs (outer lifetime)
        a1pool = ctx.enter_context(tc.tile_pool(name="a1", bufs=FT))
        a2pool = ctx.enter_context(tc.tile_pool(name="a2", bufs=FT))
        ypool = ctx.enter_context(tc.tile_pool(name="y", bufs=FT))

        # ---------- helpers -------------------------------------------------
        def proj_fm(w_name, w, act_tiles, n_tok, bias_name, out_pool, scale=None):
            """feature-major out tiles [FT x [128, n_tok]] bf16 = W.T @ act + b"""
            outs = []
            nsub = (n_tok + 511) // 512
            with tc.tile_pool(name=f"ps_{w_name}", bufs=4, space=PSUM) as pp:
                for m in range(FT):
                    ot = out_pool.tile([128, n_tok], BF, name=f"o_{w_name}_{m}",
                                       tag=f"o_{w_name}_{m}", bufs=1)
                    for ns in range(nsub):
                        c0, c1 = ns * 512, min((ns + 1) * 512, n_tok)
                        ps = pp.tile([128, c1 - c0], F32, tag="ps")
                        for k in range(FT):
                            nc.tensor.matmul(ps[:], w(k, m * 128, (m + 1) * 128),
                                             act_tiles[k][:, c0:c1],
                                             start=(k == 0), stop=(k == FT - 1))
                        if scale is not None:
                            nc.vector.tensor_scalar(ot[:, c0:c1], ps[:], scale,
                                                    bias_col(bias_name, m), MULT, ADD)
                        else:
                            nc.scalar.activation(ot[:, c0:c1], ps[:], IDENT,
                                                 bias=bias_col(bias_name, m))
                    outs.append(ot)
            return outs

        def proj_tm(w_name, w, act_tiles, bias_bcast, out_pool):
            """token-major V tiles [KT x [128, H*65]] bf16 = act.T @ W + b, with
            a ones column appended after each head's 64 dims so the AV matmul's
            65th output row is the softmax denominator for free."""
            outs = []
            with tc.tile_pool(name=f"ps_{w_name}", bufs=4, space=PSUM) as pp:
                for m in range(KT):
                    ot = out_pool.tile([128, H * 65], BF, name=f"o_{w_name}_{m}",
                                       tag=f"o_{w_name}_{m}", bufs=1)
                    ones_cols = ot.rearrange("p (h c) -> p h c", c=65)[:, :, 64:65]
                    nc.gpsimd.memset(ones_cols, 1.0)
                    for ns in range(2):
                        c0, c1 = ns * 512, (ns + 1) * 512
                        ps = pp.tile([128, 512], F32, tag="ps")
                        for k in range(FT):
                            nc.tensor.matmul(ps[:], act_tiles[k][:, m * 128:(m + 1) * 128],
                                             w(k, c0, c1),
                                             start=(k == 0), stop=(k == FT - 1))
                        dst = ot[:, ns * 8 * 65:(ns * 8 + 8) * 65].rearrange(
                            "p (h c) -> p h c", c=65)[:, :, 0:64]
                        psv = ps.rearrange("p (h c) -> p h c", c=64)
                        if bias_bcast is None:
                            copy_out(dst, psv, m * 2 + ns)
                        else:
                            bbv = bias_bcast[:, c0:c1].rearrange("p (h c) -> p h c", c=64)
                            nc.vector.scalar_tensor_tensor(dst, psv, 1.0, bbv, MULT, ADD)
                    outs.append(ot)
            return outs

        def attention(q_pairs, k_tiles, v_tiles, mask_tiles, tag, out_pool):
            """q_pairs feature-major [FT x [128,TOK]] bf16; k_tiles [FT x [128,S]];
            v_tiles token-major [KT x [128,H*65]]; mask_tiles = exp(mask) bf16
            doubled [KT x [128,2*TOK]] or None.
            Both heads of a feature pair are processed together: one [128,2*TOK]
            scoresT psum per k-tile -> exp (ACT, direct from psum) -> optional
            bf16 mask multiply (DVE 4x mode); AV accumulates per head."""
            outs = []
            with tc.tile_pool(name=f"exp_{tag}", bufs=3) as epool, \
                 tc.tile_pool(name=f"asm_{tag}", bufs=2) as spool, \
                 tc.tile_pool(name=f"sT_{tag}", bufs=2, space=PSUM) as sps, \
                 tc.tile_pool(name=f"rb_{tag}", bufs=1, space=PSUM) as rbp, \
                 tc.tile_pool(name=f"av_{tag}", bufs=3, space=PSUM) as avs:

                def finalize(av0, av1, at):
                    """softmax normalize: one combined recip row, one [64,512]
                    broadcast matmul, one copy, two muls. Deferred into the
                    NEXT head-pair's k-loop so the in-order PE never waits on
                    the DVE reciprocal."""
                    rec = spool.tile([1, 2 * TOK], BF, tag="recip")
                    with nc.allow_low_precision(reason="softmax 1/sum in bf16; output is bf16"):
                        nc.vector.reciprocal(rec[:, 0:TOK], av0[64:65, :])
                        nc.vector.reciprocal(rec[:, TOK:], av1[64:65, :])
                    rb = rbp.tile([64, 2 * TOK], F32, tag="rb")
                    nc.tensor.matmul(rb[:], ones_row_bf[0:1, 0:64], rec[:],
                                     start=True, stop=True)
                    rb_sb = spool.tile([64, 2 * TOK], F32, tag="rb_sb")
                    nc.vector.tensor_copy(rb_sb[:], rb[:])
                    nc.vector.tensor_mul(at[0:64, :], av0[0:64, :], rb_sb[:, 0:TOK])
                    nc.vector.tensor_mul(at[64:128, :], av1[0:64, :], rb_sb[:, TOK:])

                pend = None
                for hp in range(FT):
                    at = out_pool.tile([128, TOK], BF, name=f"at_{tag}_{hp}",
                                       tag=f"at_{hp}", bufs=1)
                    av0 = avs.tile([65, TOK], F32, tag="av")
                    av1 = avs.tile([65, TOK], F32, tag="av")
                    for kt in range(KT):
                        # two heads' scoresT in one 2-bank psum tile (one matmul
                        # group per bank)
                        sTp = sps.tile([128, 4 * TOK], F32, tag="sT")
                        for hh in range(2):
                            po = hh * 64
                            nc.tensor.matmul(
                                sTp[:, hh * 2 * TOK:hh * 2 * TOK + TOK],
                                k_tiles[hp][po:po + 64, kt * 128:(kt + 1) * 128],
                                q_pairs[hp][po:po + 64, :], start=True, stop=True)
                        sview = sTp.rearrange("p (b c) -> p b c", c=2 * TOK)[:, :, 0:TOK]
                        ex = epool.tile([128, 2 * TOK], BF, tag="exp")
                        exv = ex.rearrange("p (b c) -> p b c", c=TOK)
                        nc.scalar.activation(exv, sview, mybir.ActivationFunctionType.Exp)
                        if mask_tiles is not None:
                            exm = epool.tile([128, 2 * TOK], BF, tag="exm")
                            nc.vector.tensor_mul(exm[:], ex[:], mask_tiles[kt])
                            ex = exm
                        if kt == 1 and pend is not None:
                            finalize(*pend)
                            pend = None
                        for hh, av in ((0, av0), (1, av1)):
                            h = 2 * hp + hh
                            nc.tensor.matmul(av[:],
                                             v_tiles[kt][:, h * 65:(h + 1) * 65],
                                             ex[:, hh * TOK:(hh + 1) * TOK],
                                             start=(kt == 0), stop=(kt == KT - 1))
                    pend = (av0, av1, at)
                    outs.append(at)
                finalize(*pend)
            return outs

        def o_proj_residual(w_name, w, attn_tiles, bo_name, resid_tiles, rpool):
            outs = []
            with tc.tile_pool(name=f"ps_{w_name}", bufs=4, space=PSUM) as pp:
                for m in range(FT):
                    ps = pp.tile([128, TOK], F32, tag="ps")
                    for k in range(FT):
                        nc.tensor.matmul(ps[:], w(k, m * 128, (m + 1) * 128),
                                         attn_tiles[k][:], start=(k == 0), stop=(k == FT - 1))
                    rt = rpool.tile([128, TOK], BF, name=f"r_{w_name}_{m}", tag=f"r{m}")
                    nc.vector.scalar_tensor_tensor(rt[:], ps[:], bias_col(bo_name, m),
                                                   resid_tiles[m][:], ADD, ADD)
                    outs.append(rt)
            return outs

        def layer_norm(r_tiles, g_name, b_name, out_dtype, out_pool, want_bf16):
            """rstd via exp(-0.5*ln(var+eps)) so the ACT table set
            (natural_log_exp_and_others: ln/exp/relu/identity) never swaps.
            The per-k elementwise chains alternate DVE / GpSimd so the two
            engines halve the serial latency."""
            with tc.tile_pool(name=f"lnp_{g_name}", bufs=1, space=PSUM) as lnps, \
                 tc.tile_pool(name=f"lnb_{g_name}", bufs=1, space=PSUM) as lnbc, \
                 tc.tile_pool(name=f"lns_{g_name}", bufs=2) as lnsm, \
                 tc.tile_pool(name=f"lnq_{g_name}", bufs=2) as sqp:
                s1 = lnps.tile([1, TOK], F32, tag="s1")
                s2 = lnps.tile([1, TOK], F32, tag="s2")
                for k in range(FT):
                    nc.tensor.matmul(s1[:], ones_col_bf[:], r_tiles[k][:],
                                     start=(k == 0), stop=(k == FT - 1))
                for k in range(FT):
                    eng = nc.vector if k % 2 == 0 else nc.gpsimd
                    sq = sqp.tile([128, TOK], BF, tag=f"sq{k % 2}")
                    eng.tensor_mul(sq[:], r_tiles[k][:], r_tiles[k][:])
                    nc.tensor.matmul(s2[:], ones_col_bf[:], sq[:],
                                     start=(k == 0), stop=(k == FT - 1))
                mean = lnsm.tile([1, TOK], F32, tag="mean")
                nc.vector.tensor_scalar_mul(mean[:], s1[:], 1.0 / D)
                var = lnsm.tile([1, TOK], F32, tag="var")
                # var = s2/D - mean^2  ==  (s2 * 1/D) + (-mean*mean)
                nc.vector.scalar_tensor_tensor(var[:], mean[:], -1.0, mean[:], MULT, MULT)
                nc.vector.scalar_tensor_tensor(var[:], s2[:], 1.0 / D, var[:], MULT, ADD)
                lv = lnsm.tile([1, TOK], F32, tag="lv")
                nc.scalar.activation(lv[:], var[:], mybir.ActivationFunctionType.Ln,
                                     bias=eps_t[:])
                rstd = lnsm.tile([1, TOK], F32, tag="rstd")
                nc.scalar.activation(rstd[:], lv[:], mybir.ActivationFunctionType.Exp,
                                     scale=neghalf_t[:])
                mean_p = lnbc.tile([128, TOK], F32, tag="meanb")
                nc.tensor.matmul(mean_p[:], ones_row[:], mean[:], start=True, stop=True)
                rstd_p = lnbc.tile([128, TOK], F32, tag="rstdb")
                nc.tensor.matmul(rstd_p[:], ones_row[:], rstd[:], start=True, stop=True)
                mean_b = lnsm.tile([128, TOK], BF, tag="meanb_sb")
                nc.vector.tensor_copy(mean_b[:], mean_p[:])
                rstd_b = lnsm.tile([128, TOK], BF, tag="rstdb_sb")
                nc.scalar.activation(rstd_b[:], rstd_p[:], IDENT)
                outs, outs_bf = [], []
                for k in range(FT):
                    eng = nc.vector if k % 2 == 0 else nc.gpsimd
                    xn = sqp.tile([128, TOK], BF, tag=f"xn{k % 2}")
                    eng.tensor_sub(xn[:], r_tiles[k][:], mean_b[:])
                    ot = out_pool.tile([128, TOK], out_dtype, name=f"ln_{g_name}_{k}",
                                       tag=f"ln_{k}", bufs=1)
                    if zero_ln:
                        eng.tensor_mul(ot[:], xn[:], rstd_b[:])
                    else:
                        xn2 = sqp.tile([128, TOK], BF, tag=f"x2{k % 2}")
                        eng.tensor_mul(xn2[:], xn[:], rstd_b[:])
                        eng.tensor_scalar(ot[:], xn2[:], bias_col(g_name, k),
                                          bias_col(b_name, k), MULT, ADD)
                    outs.append(ot)
            return outs, outs_bf

        # ================= phase 1-3: attention blocks ======================
        warena = ctx.enter_context(tc.tile_pool(name="warena", bufs=4))
        _arena['pool'] = warena

        with tc.tile_pool(name="kvc", bufs=1) as kvc_pool:
            with tc.tile_pool(name="kvs", bufs=1) as kvs_pool, \
                 tc.tile_pool(name="mask", bufs=1) as mask_pool, \
                 tc.tile_pool(name="acts_x0", bufs=1) as actp:
                # phase 1: all Q/K/V projections (self then cross). DMA issue
                # order tracks need order: the q chunk of x0 + sWq first (the
                # very first matmuls), then the rest of x0 + sWk, etc.
                x0_b = actp.tile([128, FT, S], BF, name="x0_b", tag="x0")
                x0r = g['x0fm'].rearrange("(a p) t -> p a t", p=128)
                nc.sync.dma_start(x0_b[:, :, 0:TOK], x0r[:, :, 0:TOK])
                w_sWq = arena_load_w('sWq')
                nc.sync.dma_start(x0_b[:, 0:4, TOK:], x0r[:, 0:4, TOK:])
                nc.gpsimd.dma_start(x0_b[:, 4:8, TOK:], x0r[:, 4:8, TOK:])
                w_sWk = arena_load_w('sWk')
                w_sWv = arena_load_w('sWv')
                w_cWk = arena_load_w('cWk')
                x0_t = [x0_b[:, k, :] for k in range(FT)]
                x0cb = [x0_b[:, k, 0:TOK] for k in range(FT)]
                with tc.tile_pool(name="acts_enc", bufs=1) as ectp:
                    enc_b = ectp.tile([128, FT, S], BF, name="enc_b", tag="enc")
                    encr = g['encfm'].rearrange("(a p) t -> p a t", p=128)
                    nc.sync.dma_start(enc_b[:, 0:4, :], encr[:, 0:4, :])
                    nc.gpsimd.dma_start(enc_b[:, 4:8, :], encr[:, 4:8, :])
                    enc_t = [enc_b[:, k, :] for k in range(FT)]

                    q_self = proj_fm('sWq', w_sWq, x0cb, TOK, 'sbq', kvs_pool, scale=0.125)
                    k_self = proj_fm('sWk', w_sWk, x0_t, S, 'sbk', kvs_pool)
                    w_cWv = arena_load_w('cWv')       # reuses sWq slot
                    v_self = proj_tm('sWv', w_sWv, x0_t, free_bias['sbv'], kvs_pool)
                    w_sWo = arena_load_w('sWo')       # reuses sWk slot
                    k_cross = proj_fm('cWk', w_cWk, enc_t, S, 'cbk', kvc_pool)
                    w_cWq = arena_load_w('cWq')       # reuses sWv slot
                    v_cross = proj_tm('cWv', w_cWv, enc_t, free_bias['cbv'], kvc_pool)
                    w_cWo = arena_load_w('cWo')       # reuses cWk slot

                m01_t = None
                if g['m01'] is not None:
                    mt_b = mask_pool.tile([128, KT, 2 * TOK], BF, name="mt_b", tag="mt")
                    nc.gpsimd.dma_start(mt_b[:], g['m01'].rearrange("(a p) t -> p a t", p=128)[:])
                    m01_t = [mt_b[:, k, :] for k in range(KT)]

                phase_gate(1)
                # phase 2: self attention + O-proj + LN1 (residual = bf16
                # x0_b chunk slice)
                with tc.tile_pool(name="at_s", bufs=1) as at_pool_s, \
                     tc.tile_pool(name="r1p", bufs=1) as r1_pool:
                    attn1 = attention(q_self, k_self, v_self, m01_t, "s", at_pool_s)
                    r1 = o_proj_residual('sWo', w_sWo, attn1, 'sbo', x0cb, r1_pool)
                    a1, _ = layer_norm(r1, 'ln1_g', 'ln1_b', BF, a1pool, False)

            phase_gate(2)
            # phase 3: cross attention + O-proj + LN2; fW1 column-blocks
            # stream into the arena during the attention.
            fw_tiles = []

            def load_fw(idx):
                if idx < 4:
                    wt = arena_load(g['fW1S'].rearrange(
                        "p (j a d) -> p j a d", j=4, a=FT)[:, idx])
                else:
                    wt = arena_load(g['fW2'].rearrange(
                        "(j a p) d -> p j a d", p=128, a=FT)[:, idx - 4])
                fw_tiles.append(wt)

            with tc.tile_pool(name="qc", bufs=1) as qc_pool, \
                 tc.tile_pool(name="maskc", bufs=1) as maskc_pool, \
                 tc.tile_pool(name="r2p", bufs=1) as r2_pool:
                m01c_t = None
                if g['m01c'] is not None:
                    mtc_b = maskc_pool.tile([128, KT, 2 * TOK], BF, name="mtc_b", tag="mtc")
                    nc.gpsimd.dma_start(mtc_b[:], g['m01c'].rearrange("(a p) t -> p a t", p=128)[:])
                    m01c_t = [mtc_b[:, k, :] for k in range(KT)]
                q_cross = proj_fm('cWq', w_cWq, a1, TOK, 'cbq', qc_pool, scale=0.125)
                for j in range(4):
                    load_fw(j)
                with tc.tile_pool(name="at_c", bufs=1) as at_pool_c:
                    attn2 = attention(q_cross, k_cross, v_cross, m01c_t, "c", at_pool_c)
                    r2 = o_proj_residual('cWo', w_cWo, attn2, 'cbo', a1, r2_pool)
                a2, _ = layer_norm(r2, 'ln2_g', 'ln2_b', BF, a2pool, False)

            phase_gate(3)

        # ================= phase 4: FFN + LN3 ===============================
        # (kvc released; the vocab streaming pools open on the right side so
        # the first Wout chunks can land during the FFN.)
        wvp = ctx.enter_context(tc.tile_pool(name="wvs", bufs=9, side="right"))
        vos = ctx.enter_context(tc.tile_pool(name="vout", bufs=2, side="right"))
        PRE = 6
        chunks = {}

        def load_chunk(n):
            wt = wvp.tile([128, FT * VC], BF, name=f"wv_{n}", tag="wv")
            nc.gpsimd.dma_start(wt[:], g['WoutS'][:, n * FT * VC:(n + 1) * FT * VC])
            chunks[n] = wt

        # FFN1 m-outer over column-blocks j (fW1S block j serves m-tiles
        # 8j..8j+7); fW2 row-blocks stream into the freed arena slots and
        # FFN2 runs k-outer so block b is consumed as soon as it lands.
        with tc.tile_pool(name="hid", bufs=1) as hpool:
            h_tiles = []
            with tc.tile_pool(name="ps_f1", bufs=3, space=PSUM) as pp1:
                for m in range(HT):
                    j = m // FT
                    w1j = fw_tiles[j]
                    ps = pp1.tile([128, TOK], F32, tag="ps")
                    for k in range(FT):
                        nc.tensor.matmul(
                            ps[:], w1j[:, k, (m % FT) * 128:(m % FT + 1) * 128],
                            a2[k][:], start=(k == 0), stop=(k == FT - 1))
                    ht = hpool.tile([128, TOK], BF, name=f"h_{m}", tag=f"h_{m}")
                    nc.scalar.activation(ht[:], ps[:], mybir.ActivationFunctionType.Relu,
                                         bias=fb1_sb[:, m:m + 1])
                    h_tiles.append(ht)
                    if m % FT == FT - 1:
                        load_fw(4 + j)  # fW2 row-block into freed slot
            for n in range(PRE):
                load_chunk(n)
            r3 = []
            with tc.tile_pool(name="r3p", bufs=1) as r3_pool:
                with tc.tile_pool(name="ps_f2", bufs=1, space=PSUM) as pp2:
                    ps2 = [pp2.tile([128, TOK], F32, name=f"psf2_{m}", tag=f"ps{m}")
                           for m in range(FT)]
                    for b in range(4):
                        w2b = fw_tiles[4 + b]
                        for kk in range(FT):
                            kh = b * FT + kk
                            for m in range(FT):
                                nc.tensor.matmul(ps2[m][:], w2b[:, kk, m * 128:(m + 1) * 128],
                                                 h_tiles[kh][:], start=(kh == 0),
                                                 stop=(kh == HT - 1))
                    for m in range(FT):
                        rt = r3_pool.tile([128, TOK], BF, name=f"r_ffn_{m}", tag=f"r{m}")
                        nc.vector.scalar_tensor_tensor(rt[:], ps2[m][:], bias_col('fb2', m),
                                                       a2[m][:], ADD, ADD)
                        r3.append(rt)
                y, _ = layer_norm(r3, 'ln3_g', 'ln3_b', BF, ypool, False)

        phase_gate(4)
        # ================= phase 5: streamed full-vocab projection ==========
        # out[tok, v] = y.T @ Wout ; y tiles are the lhsT directly. Chunk
        # DMAs software-pipelined PRE deep on the gpsimd/vector SWDGE queues.
        with tc.tile_pool(name="vps", bufs=4, space=PSUM) as vps, \
             tc.tile_pool(name="bps", bufs=2, space=PSUM) as bps, \
             tc.tile_pool(name="brow", bufs=1) as brp:
            bout_sb = None
            if not zero_free_biases:
                bout_sb = brp.tile([1, V], F32, name="bout_sb", tag="br")
                nc.sync.dma_start(bout_sb[:], g['bout_row'][:])
            for gi in range(VN // VG):
                ot = vos.tile([128, 2, VG * VC], BF, tag="vo")
                for nn in range(VG):
                    n = gi * VG + nn
                    wt = chunks.pop(n)
                    if n + PRE < VN:
                        load_chunk(n + PRE)
                    wv = wt.rearrange("p (a c) -> p a c", c=VC)
                    bb = None
                    if bout_sb is not None:
                        bb = bps.tile([128, VC], F32, tag="bb")
                        nc.tensor.matmul(bb[:], ones_row[:],
                                         bout_sb[0:1, n * VC:(n + 1) * VC],
                                         start=True, stop=True)
                    for m in range(2):
                        ps = vps.tile([128, VC], F32, tag="ps")
                        for k in range(FT):
                            nc.tensor.matmul(ps[:], y[k][:, m * 128:(m + 1) * 128],
                                             wv[:, k, :],
                                             start=(k == 0), stop=(k == FT - 1))
                        dst = ot[:, m, nn * VC:(nn + 1) * VC]
                        if bb is None:
                            copy_out(dst, ps[:], n * 2 + m)
                        else:
                            nc.vector.scalar_tensor_tensor(dst, ps[:], 1.0, bb[:],
                                                           MULT, ADD)
                for m in range(2):
                    eng = nc.sync if m == 0 else nc.scalar
                    eng.dma_start(
                        g['out'][m * 128:(m + 1) * 128, gi * VG * VC:(gi + 1) * VG * VC],
                        ot[:, m, :])


def host_prep(inputs):
    x0 = np.asarray(inputs['dec_input'], np.float32) + positional_encoding(S, D)[None]
    enc = np.asarray(inputs['enc_input'], np.float32)
    mask_self = np.asarray(inputs['masked_attention_mask'], np.float32)[0, 0]
    mask_cross = np.asarray(inputs['cross_attention_mask'], np.float32)[0, 0]
    self_mul = bool(np.any(mask_self != 0.0))
    cross_mul = bool(np.any(mask_cross != 0.0))
    li = L - 1
    Wl = {}
    for p in ['sWq', 'sWk', 'sWv', 'sWo', 'cWq', 'cWk', 'cWv', 'cWo', 'fW2']:
        Wl[p] = np.ascontiguousarray(np.asarray(inputs[p], np.float32)[li]).astype(BF16)
    # fW1 column-blocks: fW1S[p, j*FT*1024 + a*1024 + d] = fW1[a*128+p, j*1024+d]
    f1 = np.asarray(inputs['fW1'], np.float32)[li]
    W1c = f1.reshape(FT, 128, 4, 1024)
    Wl['fW1S'] = np.ascontiguousarray(
        W1c.transpose(1, 2, 0, 3).reshape(128, 4 * FT * 1024)).astype(BF16)
    bv = {}
    for p in ['sbq', 'sbk', 'sbv', 'sbo', 'cbq', 'cbk', 'cbv', 'cbo',
              'ln1_g', 'ln1_b', 'ln2_g', 'ln2_b', 'ln3_g', 'ln3_b', 'fb1', 'fb2']:
        bv[p] = np.asarray(inputs[p], np.float32)[li]
    Wout = np.asarray(inputs['Wout'], np.float32)
    bout = np.asarray(inputs['bout'], np.float32)

    # chunk-major Wout: WoutS[p, n*FT*VC + a*VC + c] = Wout[a*128+p, n*VC+c]
    Wc = Wout.reshape(FT, 128, VN, VC)
    WoutS = np.ascontiguousarray(Wc.transpose(1, 2, 0, 3).reshape(128, VN * FT * VC)).astype(BF16)

    def pp(v):  # [1024] -> [128, 8] partition-major
        return np.ascontiguousarray(v.reshape(-1, 128).T)

    bias_cols = []
    for name in BIAS_NAMES:
        src = {'sbq': bv['sbq'] * 0.125, 'cbq': bv['cbq'] * 0.125}.get(name, bv.get(name))
        bias_cols.append(pp(src))
    biases_pp = np.ascontiguousarray(np.concatenate(bias_cols, axis=1), np.float32)
    fb1_pp = np.ascontiguousarray(bv['fb1'].reshape(HT, 128).T, np.float32)

    in_maps = []
    for core in range(NC):
        b, c = core // 4, core % 4
        q0 = c * TOK
        rot = np.concatenate([np.arange(q0, S), np.arange(0, q0)])
        m = {
            'x0fm': np.ascontiguousarray(x0[b][rot].T).astype(BF16),
            'encfm': np.ascontiguousarray(enc[b].T).astype(BF16),
            'biases': biases_pp, 'fb1': fb1_pp,
            'sbv_row': np.ascontiguousarray(bv['sbv'][None, :], np.float32),
            'cbv_row': np.ascontiguousarray(bv['cbv'][None, :], np.float32),
            'bout_row': np.ascontiguousarray(bout[None, :], np.float32),
            'WoutS': WoutS,
        }
        m.update(Wl)
        if self_mul:
            # mask rows = this core's queries, columns in the same rotated
            # key order as x0fm
            mt = np.exp(np.minimum(mask_self[q0:q0 + TOK][:, rot].T, 80.0))
            m['m01'] = np.ascontiguousarray(
                np.concatenate([mt, mt], axis=1)).astype(BF16)
        if cross_mul:
            mt = np.exp(np.minimum(mask_cross[q0:q0 + TOK, :].T, 80.0))
            m['m01c'] = np.ascontiguousarray(
                np.concatenate([mt, mt], axis=1)).astype(BF16)
        in_maps.append(m)
    zero_free = not (np.any(bv['sbv']) or np.any(bv['cbv']) or np.any(bout))
    zero_ln = all(np.all(bv[f'ln{i}_g'] == 1.0) and not np.any(bv[f'ln{i}_b'])
                  for i in (1, 2, 3))
    return in_maps, self_mul, cross_mul, zero_free, zero_ln


_CACHE = {}


def _get_program(self_mul, cross_mul, zero_free, zero_ln):
    key = (self_mul, cross_mul, zero_free, zero_ln)
    if key not in _CACHE:
        _CACHE[key] = build_program(self_mul, cross_mul, zero_free, zero_ln)
    return _CACHE[key]


def kernel(**inputs):
    in_maps, self_mul, cross_mul, zero_free, zero_ln = host_prep(inputs)
    nc = _get_program(self_mul, cross_mul, zero_free, zero_ln)
    res = run_bass_kernel_spmd(nc, in_maps, core_ids=list(range(NC)))
    full = np.empty((B, S, V), np.float32)
    for r in range(NC):
        b, c = r // 4, r % 4
        full[b, c * TOK:(c + 1) * TOK, :] = np.asarray(res.results[r]["out"], np.float32)
    return full
